# revision 31
# baseline (speedup 1.0000x reference)
"""Causal self-attention (b=2, t=2048, d_model=1024, 16 heads) on 8 trn2 cores.

Sharding: tensor-parallel over heads (2 heads per core). Each core computes
qkv = x @ W_qkv[:, head-slice], attention for its heads, and a partial
out_heads @ W_proj[head-rows, :]. The 8 partial [4096, 1024] fp16 outputs are
summed on the host (the all-reduce after proj), plus b_proj.

Device pipeline (per 512-row chunk, engines kept co-busy):
- Stage A: Q^T/K^T = W.T @ xT on PE (column-major, heads stacked on
  partitions); V computed ROW-major directly (lhsT = xT tile, rhs = W_v
  k-tile) so no PE transposes / DVE copies are needed — V lands in PSUM as
  [rows, dims] and Pool evacuates it into the 130-wide V blocks (ones column
  per head accumulates the softmax denominator during the AV matmul).
- Scores are computed transposed (sT[k, q]); no max-subtraction needed.
- Causality: k-tiles above the diagonal are skipped; diagonal 128-bands get
  -2000 added in PSUM via a small accumulate-matmul (exp then gives 0),
  removing the DVE mask-multiply from the exp->AV critical chain.
- Softmax normalize: one DVE reciprocal of the accumulated denominator row,
  one PE broadcast matmul (ones[1,128] lhsT), two DVE multiplies -> ot.
- Proj per 128-row tile into PSUM; Pool evacuates to fp16; one DMA per
  512-row chunk writes the fp16 partial to HBM (host sums partials in f32).
- PSUM budget: 2 x [128,1024] score/proj slots + 2 x [65,1024] AV-accum
  slots = 8 banks. Normalize+proj of chunk c are emitted interleaved into
  chunk c+1's k-loop so PE/ACT never drain.
"""

import sys

sys.path.insert(0, "/opt/trn_rl_repo")

import numpy as np

import concourse.bass as bass  # noqa: F401
import concourse.tile as tile
from concourse import bacc, mybir

F32 = mybir.dt.float32
F32R = mybir.dt.float32r
F16 = mybir.dt.float16
F8 = mybir.dt.float8e4
BF16 = mybir.dt.bfloat16
DT_AT = BF16   # operand dtype for stage A matmuls and attention (B/C)
EXP = mybir.ActivationFunctionType.Exp
IDENT = mybir.ActivationFunctionType.Identity

B = 2
T = 2048
DM = 1024
NH = 16
HD = 64
ROWS = B * T            # 4096
NCORES = 8
HPC = NH // NCORES      # heads per core = 2
WCOLS = HPC * HD        # 128 qkv columns per core for each of q/k/v
QCH = 512               # query chunk
KTILE = 128             # key tile
NQC = T // QCH          # 4 query chunks per batch
NKT_B = T // KTILE      # 16 key tiles per batch
NRC = ROWS // QCH       # 8 row chunks
NKD = DM // 128         # 8 d_model k-tiles
VW = 2 * (HD + 1)       # 130: V block width (2 heads x (64 dims + ones col))
MASKV = -2000.0         # pre-scale causal mask add; exp(0.125*-2000) == 0


class _Alloc:
    """Tag-based routing to the right tile pool."""
    WORK = {"xt", "ea", "osb", "rc2", "bc", "ea8"}
    WORK_BUFS = {"xt": 3, "ea": 8, "osb": 3, "rc2": 2, "bc": 2, "ea8": 4}

    def __init__(self, pers, work, ps):
        self.pers, self.work, self.ps = pers, work, ps

    def tile(self, shape, dt, tag):
        if tag in ("ps2", "pso"):
            return self.ps.tile(shape, dt, tag=tag, name=tag, bufs=2)
        if tag in self.WORK:
            return self.work.tile(shape, dt, tag=tag, name=tag, bufs=self.WORK_BUFS[tag])
        return self.pers.tile(shape, dt, tag=tag, name=tag)


def _emit_consts(nc, al, aps):
    (xt_d, wq_d, wk_d, wv_d, wp_d, wpb_d, bq_d, bk_d, bvr_d, caus_d, causf_d,
     vones8_d, onesb_d, nl8_d, onesr_d, id_d, vones_d, out_d) = aps
    C = {}
    C["qt"] = al.tile([128, ROWS], DT_AT, tag="qt")
    C["kt"] = al.tile([128, ROWS], DT_AT, tag="kt")
    C["v"] = al.tile([128, (ROWS // 128) * VW], DT_AT, tag="v")
    C["ot"] = al.tile([128, ROWS], F32R, tag="ot")
    C["wq"] = al.tile([128, DM], DT_AT, tag="wq")
    C["wk"] = al.tile([128, DM], DT_AT, tag="wk")
    C["wv"] = al.tile([128, DM], DT_AT, tag="wv")
    C["wp"] = al.tile([128, DM], F32R, tag="wp")
    C["wpb"] = al.tile([128, DM], DT_AT, tag="wpb")
    C["otb"] = al.tile([128, ROWS], DT_AT, tag="otb")
    C["bq"] = al.tile([128, 1], F32, tag="bq")
    C["bk"] = al.tile([128, 1], F32, tag="bk")
    C["bvr"] = al.tile([1, 128], DT_AT, tag="bvr")
    C["caus"] = al.tile([128, 128], DT_AT, tag="caus")
    C["causf"] = al.tile([128, 128], DT_AT, tag="causf")
    C["v8a"] = al.tile([128, (ROWS // 128) * (HD + 1)], F8, tag="v8a")
    C["v8b"] = al.tile([128, (ROWS // 128) * (HD + 1)], F8, tag="v8b")
    C["ones_b"] = al.tile([1, 128], DT_AT, tag="ones_b")
    C["nl8"] = al.tile([128, 1], F32, tag="nl8")
    C["ones_r"] = al.tile([1, 128], F32R, tag="ones_r")
    C["id"] = al.tile([128, 128], DT_AT, tag="id")
    for k in range(NKD):
        nc.sync.dma_start(C["wq"][:, k * 128:(k + 1) * 128], wq_d[k * 128:(k + 1) * 128, :])
        nc.sync.dma_start(C["wk"][:, k * 128:(k + 1) * 128], wk_d[k * 128:(k + 1) * 128, :])
        nc.sync.dma_start(C["wv"][:, k * 128:(k + 1) * 128], wv_d[k * 128:(k + 1) * 128, :])
    nc.sync.dma_start(C["wp"][:], wp_d[:])
    nc.sync.dma_start(C["wpb"][:], wpb_d[:])
    nc.sync.dma_start(C["bq"][:], bq_d[:])
    nc.sync.dma_start(C["bk"][:], bk_d[:])
    nc.sync.dma_start(C["bvr"][:], bvr_d[:])
    nc.sync.dma_start(C["caus"][:], caus_d[:])
    nc.sync.dma_start(C["causf"][:], causf_d[:])
    v8a_blocks = C["v8a"].rearrange("p (i w) -> p i w", w=HD + 1)
    v8b_blocks = C["v8b"].rearrange("p (i w) -> p i w", w=HD + 1)
    nc.sync.dma_start(v8a_blocks[:, :, HD], vones8_d[:])
    nc.sync.dma_start(v8b_blocks[:, :, HD], vones8_d[:])
    nc.sync.dma_start(C["ones_b"][:], onesb_d[:])
    nc.sync.dma_start(C["nl8"][:], nl8_d[:])
    nc.sync.dma_start(C["ones_r"][:], onesr_d[:])
    nc.sync.dma_start(C["id"][:], id_d[:])
    # ones columns of the V blocks (cols 64 and 129 of each 130-block)
    v_blocks = C["v"].rearrange("p (i w) -> p i w", w=VW)
    nc.sync.dma_start(v_blocks[:, :, HD], vones_d[:])
    nc.sync.dma_start(v_blocks[:, :, 2 * HD + 1], vones_d[:])
    return C


def _emit_stage_a(nc, al, aps, C, opts=frozenset()):
    (xt_d, wq_d, wk_d, wv_d, wp_d, wpb_d, bq_d, bk_d, bvr_d, caus_d, causf_d,
     vones8_d, onesb_d, nl8_d, onesr_d, id_d, vones_d, out_d) = aps
    qt_sb, kt_sb, v_sb = C["qt"], C["kt"], C["v"]
    wq_sb, wk_sb, wv_sb = C["wq"], C["wk"], C["wv"]

    for rc in range(NRC):
        cs = rc * QCH
        xt_t = al.tile([128, NKD * QCH], DT_AT, tag="xt")
        xt_v = xt_t.rearrange("p (k q) -> p k q", q=QCH)
        if "noxt" not in opts:
            nc.sync.dma_start(
                xt_v,
                xt_d[:, cs:cs + QCH].rearrange("(k p) q -> p k q", p=128))
        qk = al.tile([128, 2 * QCH], F32, tag="ps2")
        psq = qk[:, 0:QCH]
        psk = qk[:, QCH:2 * QCH]
        psv = al.tile([128, QCH], F32, tag="pso")
        # The 4 V row-blocks are independent sub-bank accumulations; PSUM
        # allows only one start-group per 2KB zero region, so pre-zero the
        # bank and accumulate everything with start=False.
        nc.vector.memzero(psv[:])
        for k in range(NKD):
            st = (k == 0)
            sp = (k == NKD - 1)
            nc.tensor.matmul(psq, wq_sb[:, k * 128:(k + 1) * 128], xt_v[:, k, :], start=st, stop=sp)
            nc.tensor.matmul(psk, wk_sb[:, k * 128:(k + 1) * 128], xt_v[:, k, :], start=st, stop=sp)
            for i in range(QCH // 128):
                nc.tensor.matmul(psv[:, i * 128:(i + 1) * 128],
                                 xt_v[:, k, i * 128:(i + 1) * 128],
                                 wv_sb[:, k * 128:(k + 1) * 128],
                                 start=False, stop=False, skip_group_check=True)
        for i in range(QCH // 128):
            nc.tensor.matmul(psv[:, i * 128:(i + 1) * 128], C["ones_b"][:],
                             C["bvr"][:], start=False, stop=False,
                             skip_group_check=True)
        nc.scalar.activation(qt_sb[:, cs:cs + QCH], psq, IDENT, bias=C["bq"][:])
        nc.scalar.activation(kt_sb[:, cs:cs + QCH], psk, IDENT, bias=C["bk"][:])
        VW8 = HD + 1
        for i in range(QCH // 128):
            blk = rc * (QCH // 128) + i
            if "fp8av" in opts:
                dst_a = C["v8a"][:, blk * VW8:blk * VW8 + HD]
                dst_b = C["v8b"][:, blk * VW8:blk * VW8 + HD]
            else:
                dst_a = v_sb[:, blk * VW:blk * VW + HD]
                dst_b = v_sb[:, blk * VW + HD + 1:blk * VW + 2 * HD + 1]
            if "vact" in opts:
                nc.scalar.activation(dst_a, psv[:, i * 128:i * 128 + HD], IDENT)
                nc.scalar.activation(dst_b, psv[:, i * 128 + HD:(i + 1) * 128], IDENT)
            else:
                nc.vector.tensor_copy(dst_a, psv[:, i * 128:i * 128 + HD])
                nc.vector.tensor_copy(dst_b, psv[:, i * 128 + HD:(i + 1) * 128])


def _emit_norm(nc, al, C, qglob, pso2, opts=frozenset()):
    """Softmax normalize chunk at qglob: ot[:, qglob:+QCH] = pso/denom."""
    ot_sb = C["otb"] if "otbf" in opts else C["ot"]
    rcv = al.tile([1, 2 * QCH], F32R, tag="rc2")
    with nc.allow_low_precision(reason="f32r softmax denom recip"):
        nc.vector.reciprocal(rcv[:], pso2[HD:HD + 1, :])
    bc2 = al.tile([HD, 2 * QCH], F32, tag="bc")
    if "nobc" in opts:
        nc.gpsimd.memset(bc2[:], 1.0)  # timing ablation: skip real broadcast
    elif "dmabc" in opts:
        # broadcast the reciprocal row across 64 partitions via DMA (keeps
        # PE/ACT out of the normalize chain)
        nc.sync.dma_start(bc2[:], rcv.bitcast(F32).partition_broadcast(HD)[:, 0, :])
    else:
        psbc = al.tile([128, 2 * QCH], F32, tag="ps2")
        nc.tensor.matmul(psbc[0:HD, 0:QCH], C["ones_r"][:, 0:HD], rcv[:, 0:QCH])
        nc.tensor.matmul(psbc[0:HD, QCH:2 * QCH], C["ones_r"][:, 0:HD],
                         rcv[:, QCH:2 * QCH])
        nc.scalar.activation(bc2[:], psbc[0:HD, :], IDENT)
    nc.vector.tensor_mul(ot_sb[0:HD, qglob:qglob + QCH], pso2[0:HD, 0:QCH],
                         bc2[:, 0:QCH])
    nc.vector.tensor_mul(ot_sb[HD:128, qglob:qglob + QCH], pso2[0:HD, QCH:2 * QCH],
                         bc2[:, QCH:2 * QCH])


def _emit_proj(nc, al, aps, C, qglob, j, osb, opts=frozenset()):
    """Proj one 128-row q-tile j of the chunk at qglob into osb (fp16)."""
    if "otbf" in opts:
        ot_sb, wp_sb = C["otb"], C["wpb"]
    else:
        ot_sb, wp_sb = C["ot"], C["wp"]
    q0 = qglob + j * 128
    psp = al.tile([128, 2 * QCH], F32, tag="ps2")
    for ct in range(DM // 512):
        nc.tensor.matmul(psp[:, ct * 512:(ct + 1) * 512], ot_sb[:, q0:q0 + 128],
                         wp_sb[:, ct * 512:(ct + 1) * 512])
    if "noosb" in opts:
        return
    if j % 2 == 0:
        nc.scalar.activation(osb[:, j * DM:(j + 1) * DM], psp[:], IDENT)
    else:
        nc.vector.tensor_copy(osb[:, j * DM:(j + 1) * DM], psp[:])


def _emit_outdma(nc, aps, qglob, osb):
    out_d = aps[-1]
    nc.gpsimd.dma_start(
        out_d[qglob:qglob + QCH, :].rearrange("(j p) d -> p j d", p=128),
        osb.rearrange("p (j d) -> p j d", d=DM))


def _emit_attn(nc, al, aps, C, opts=frozenset()):
    (xt_d, wq_d, wk_d, wv_d, wp_d, wpb_d, bq_d, bk_d, bvr_d, caus_d, causf_d,
     vones8_d, onesb_d, nl8_d, onesr_d, id_d, vones_d, out_d) = aps
    qt_sb, kt_sb, v_sb = C["qt"], C["kt"], C["v"]
    caus_sb, id_sb = C["caus"], C["id"]

    NJ = QCH // 128  # 4 proj tiles per chunk

    # Two-chunk software pipeline: normalize of chunk c-1 and projection of
    # chunk c-2 are interleaved into chunk c's k-loop. By the time proj(c-2)
    # matmuls reach PE, ot(c-2) has been ready for a whole chunk, so the
    # recip->bcast->mul chain latency never gates the PE queue.
    def make_steps(h1, h2):
        steps = []
        if h1 is not None and "nonorm" not in opts and "noproj" not in opts:
            steps.append(("norm", h1))
        if h2 is not None and "noproj" not in opts:
            steps += [("proj", h2, j) for j in range(NJ)]
        return steps

    def emit_step(st):
        if st[0] == "norm":
            qg, pso2, osb = st[1]
            _emit_norm(nc, al, C, qg, pso2, opts)
        else:
            qg, pso2, osb = st[1]
            j = st[2]
            _emit_proj(nc, al, aps, C, qg, j, osb, opts)
            if j == NJ - 1 and "noout" not in opts and "noosb" not in opts:
                _emit_outdma(nc, aps, qg, osb)

    hist = [None, None]
    for b in range(B):
        # Chunk order [1,3,2,0]: qc1 first so attention starts as soon as
        # two stage-A row-chunks land; deferred norm/proj always has a long
        # successor k-loop to hide in; the drain tail is the shortest chunk.
        for qc in (1, 3, 2, 0):
            qglob = b * T + qc * QCH
            nkt = (qc + 1) * (QCH // KTILE)
            pso2 = al.tile([HD + 1, 2 * QCH], F32, tag="pso")
            osb = al.tile([128, NJ * DM], F16, tag="osb")
            steps = make_steps(hist[0], hist[1])
            step = 0
            if "fp8av" in opts:
                # Pair-based k-loop: exp scores for two adjacent k-tiles into
                # one fp8 tile, then one DoubleRow AV matmul per head
                # contracts both tiles at once (0.5 cycles/row).
                ndg = QCH // KTILE
                pairs = [(qc * ndg + 2 * j, True) for j in range(ndg // 2)] + \
                        [(2 * j, False) for j in range(qc * ndg // 2)]
                idx = 0
                for pi, (kte, isdg) in enumerate(pairs):
                    sp_ = max(0, kte * KTILE - qc * QCH)  # pair column start
                    ea8 = al.tile([128, 4 * QCH], F8, tag="ea8")
                    ea8v = ea8.rearrange("p (i h q) -> p i h q", i=2, h=2)
                    for par in range(2):
                        kt = kte + par
                        r = kt * KTILE - qc * QCH
                        i = b * NKT_B + kt
                        kcol = b * T + kt * KTILE
                        ps2 = al.tile([128, 2 * QCH], F32, tag="ps2")
                        nc.tensor.matmul(ps2[:, sp_:QCH],
                                         kt_sb[0:HD, kcol:kcol + KTILE],
                                         qt_sb[0:HD, qglob + sp_:qglob + QCH],
                                         start=True, stop=not isdg)
                        nc.tensor.matmul(ps2[:, QCH + sp_:],
                                         kt_sb[HD:128, kcol:kcol + KTILE],
                                         qt_sb[HD:128, qglob + sp_:qglob + QCH],
                                         start=True, stop=not isdg)
                        if isdg:
                            # triangular band at this tile's own diagonal
                            for off in (0, QCH):
                                if par == 1:  # fully-masked 128 cols below
                                    nc.tensor.matmul(
                                        ps2[:, off + sp_:off + sp_ + KTILE],
                                        C["causf"][:], id_sb[:],
                                        start=False, stop=False)
                                nc.tensor.matmul(
                                    ps2[:, off + r:off + r + KTILE],
                                    caus_sb[:], id_sb[:],
                                    start=False, stop=True)
                        src_v = ps2.rearrange("p (h q) -> p h q", h=2)[:, :, sp_:]
                        dst_v = ea8v[:, par, :, sp_:]
                        # -ln4 bias keeps exp outputs inside fp8-e4m3 range
                        # (max logit ~7 -> e^7/4 = 274 < 448) while limiting
                        # denormal quantization of small weights; the softmax
                        # denominator scales identically so it cancels.
                        nc.scalar.activation(dst_v, src_v, EXP, scale=0.125,
                                             bias=C["nl8"][:])
                    st = (pi == 0)
                    sp2 = (pi == len(pairs) - 1)
                    blk = b * NKT_B + kte
                    VW8 = HD + 1
                    for h, v8 in ((0, C["v8a"]), (1, C["v8b"])):
                        nc.tensor.matmul(
                            pso2[:, h * QCH + sp_:(h + 1) * QCH],
                            v8[:, blk * VW8:(blk + 2) * VW8].rearrange(
                                "p (i m) -> p i m", i=2),
                            ea8v[:, :, h, sp_:],
                            start=st, stop=sp2,
                            perf_mode=mybir.MatmulPerfMode.DoubleRow)
                    for _ in range(2):
                        if idx >= 1 and step < len(steps):
                            emit_step(steps[step])
                            step += 1
                        idx += 1
                while step < len(steps):
                    emit_step(steps[step])
                    step += 1
                hist = [(qglob, pso2, osb), hist[0]]
                continue
            # Diagonal (small, latency-bound) k-tiles first: their
            # sc->mask->exp->AV chains overlap the deferred norm/proj of the
            # previous chunk; the big full tiles then stream PE-dense.
            kts = list(range(qc * (QCH // KTILE), nkt)) + list(range(qc * (QCH // KTILE)))
            for idx, kt in enumerate(kts):
                r = kt * KTILE - qc * QCH
                s = max(0, r)          # valid column suffix start
                i = b * NKT_B + kt     # global 128-row tile index for K/V
                kcol = b * T + kt * KTILE
                ps2 = al.tile([128, 2 * QCH], F32, tag="ps2")
                diag = r >= 0 and "nomask" not in opts
                nc.tensor.matmul(ps2[:, s:QCH], kt_sb[0:HD, kcol:kcol + KTILE],
                                 qt_sb[0:HD, qglob + s:qglob + QCH],
                                 start=True, stop=not diag)
                nc.tensor.matmul(ps2[:, QCH + s:], kt_sb[HD:128, kcol:kcol + KTILE],
                                 qt_sb[HD:128, qglob + s:qglob + QCH],
                                 start=True, stop=not diag)
                if diag and "nomask" not in opts:
                    # add -2000 to the upper-triangular 128-band
                    nc.tensor.matmul(ps2[:, s:s + KTILE], caus_sb[:], id_sb[:],
                                     start=False, stop=True)
                    nc.tensor.matmul(ps2[:, QCH + s:QCH + s + KTILE], caus_sb[:],
                                     id_sb[:], start=False, stop=True)
                ea2 = al.tile([128, 2 * QCH], DT_AT, tag="ea")
                src_v = ps2.rearrange("p (h q) -> p h q", h=2)[:, :, s:]
                dst_v = ea2.rearrange("p (h q) -> p h q", h=2)[:, :, s:]
                if "noexp" in opts:
                    nc.scalar.activation(dst_v, src_v, IDENT, scale=1e-9)
                else:
                    nc.scalar.activation(dst_v, src_v, EXP, scale=0.125)
                st = (idx == 0)
                sp = (idx == nkt - 1)
                nc.tensor.matmul(pso2[:, s:QCH], v_sb[:, i * VW:i * VW + HD + 1],
                                 ea2[:, s:QCH], start=st, stop=sp)
                nc.tensor.matmul(pso2[:, QCH + s:], v_sb[:, i * VW + HD + 1:i * VW + VW],
                                 ea2[:, QCH + s:], start=st, stop=sp)
                if idx >= 1 and step < len(steps):
                    emit_step(steps[step])
                    step += 1
            while step < len(steps):
                emit_step(steps[step])
                step += 1
            hist = [(qglob, pso2, osb), hist[0]]
    # flush: norm of the last chunk, then proj of the last two chunks
    for st in make_steps(hist[0], hist[1]):
        emit_step(st)
    for st in make_steps(None, hist[0]):
        emit_step(st)


def _emit_body(nc, al, aps, C, parts=("a", "bc")):
    opts = set(parts)
    if "a" in parts:
        _emit_stage_a(nc, al, aps, C, opts)
    if "bc" in parts:
        _emit_attn(nc, al, aps, C, opts)


def build_module(repeat=1, loop_n=0, parts=("a", "bc"), pre_parts=()):
    nc = bacc.Bacc("TRN2", target_bir_lowering=False, debug=False,
                   enable_asserts=True, num_devices=NCORES)

    def din(name, shape, dt):
        return nc.dram_tensor(name, shape, dt, kind="ExternalInput").ap()

    aps = (
        din("xt", [DM, ROWS], DT_AT),
        din("wq", [DM, WCOLS], DT_AT),
        din("wk", [DM, WCOLS], DT_AT),
        din("wv", [DM, WCOLS], DT_AT),
        din("wp", [WCOLS, DM], F32R),
        din("wpb", [WCOLS, DM], DT_AT),
        din("bq", [WCOLS, 1], F32),
        din("bk", [WCOLS, 1], F32),
        din("bvr", [1, WCOLS], DT_AT),
        din("caus", [128, 128], DT_AT),
        din("causf", [128, 128], DT_AT),
        din("vones8", [128, ROWS // 128], F8),
        din("ones_b", [1, 128], DT_AT),
        din("nl8", [128, 1], F32),
        din("ones_r", [1, 128], F32R),
        din("ident", [128, 128], DT_AT),
        din("vones", [128, ROWS // 128], DT_AT),
        nc.dram_tensor("out", [ROWS, DM], F16, kind="ExternalOutput").ap(),
    )
    with tile.TileContext(nc) as tc:
        with tc.tile_pool(name="pers", bufs=1) as pers, \
             tc.tile_pool(name="work", bufs=2) as work, \
             tc.tile_pool(name="ps", bufs=2, space="PSUM") as psp:
            al = _Alloc(pers, work, psp)
            al.tc = tc
            consts = _emit_consts(nc, al, aps)
            if pre_parts:
                _emit_body(nc, al, aps, consts, parts=pre_parts)
            if loop_n:
                with tc.For_i(0, loop_n, 1):
                    for r in range(repeat):
                        _emit_body(nc, al, aps, consts, parts=parts)
            else:
                for r in range(repeat):
                    _emit_body(nc, al, aps, consts, parts=parts)
    nc.compile()
    return nc


def _host_prep(x, W_qkv, b_qkv, W_proj):
    import ml_dtypes
    bf16 = ml_dtypes.bfloat16
    x = np.asarray(x, np.float32)
    W_qkv = np.asarray(W_qkv, np.float32)
    b_qkv = np.asarray(b_qkv, np.float32)
    W_proj = np.asarray(W_proj, np.float32)
    xt = np.ascontiguousarray(x.reshape(ROWS, DM).T.astype(bf16))
    caus = np.triu(np.full((128, 128), MASKV, np.float32), 1).astype(bf16)
    f8 = mybir.dt.np(mybir.dt.float8e4)
    causf = np.full((128, 128), MASKV, np.float32).astype(bf16)
    vones8 = np.ones((128, ROWS // 128), f8)
    ident = np.eye(128, dtype=bf16)
    in_maps = []
    for c in range(NCORES):
        h0 = c * WCOLS  # first qkv column of this core's 2 heads
        in_maps.append({
            "xt": xt,
            "wq": np.ascontiguousarray(W_qkv[:, h0:h0 + WCOLS].astype(bf16)),
            "wk": np.ascontiguousarray(W_qkv[:, DM + h0:DM + h0 + WCOLS].astype(bf16)),
            "wv": np.ascontiguousarray(W_qkv[:, 2 * DM + h0:2 * DM + h0 + WCOLS].astype(bf16)),
            "wp": np.ascontiguousarray(W_proj[h0:h0 + WCOLS, :]),
            "wpb": np.ascontiguousarray(W_proj[h0:h0 + WCOLS, :].astype(bf16)),
            "bq": np.ascontiguousarray(b_qkv[h0:h0 + WCOLS, None]),
            "bk": np.ascontiguousarray(b_qkv[DM + h0:DM + h0 + WCOLS, None]),
            "bvr": np.ascontiguousarray(
                b_qkv[2 * DM + h0:2 * DM + h0 + WCOLS][None, :].astype(bf16)),
            "caus": caus,
            "causf": causf,
            "vones8": vones8,
            "ones_b": np.ones((1, 128), bf16),
            "nl8": np.full((128, 1), -np.log(4.0), np.float32),
            "ones_r": np.ones((1, 128), np.float32),
            "ident": ident,
            "vones": np.ones((128, ROWS // 128), bf16),
        })
    return in_maps


class _Runner:
    """Compile once, execute many times (mirrors bass2jax.run_bass_via_pjrt)."""

    def __init__(self, nc):
        import jax
        from jax.sharding import Mesh, PartitionSpec
        from jax.experimental.shard_map import shard_map
        from concourse import bass2jax
        from concourse import mybir as _mybir

        bass2jax.install_neuronx_cc_hook()
        self.jax = jax
        in_names, out_names, out_avals, zero_shapes = [], [], [], []
        partition_name = nc.partition_id_tensor.name if nc.partition_id_tensor else None
        for alloc in nc.m.functions[0].allocations:
            if not isinstance(alloc, _mybir.MemoryLocationSet):
                continue
            name = alloc.memorylocations[0].name
            if alloc.kind == "ExternalInput":
                if name != partition_name:
                    in_names.append(name)
            elif alloc.kind == "ExternalOutput":
                shape = tuple(alloc.tensor_shape)
                dtype = _mybir.dt.np(alloc.dtype)
                out_names.append(name)
                out_avals.append(jax.core.ShapedArray(shape, dtype))
                zero_shapes.append((shape, dtype))
        self.in_names = in_names
        self.out_names = out_names
        self.out_avals = out_avals
        self.zero_shapes = zero_shapes
        n_params = len(in_names)
        n_outs = len(out_avals)
        all_in_names = in_names + out_names + ([partition_name] if partition_name else [])

        def _body(*args):
            operands = list(args)
            if partition_name is not None:
                operands.append(bass2jax.partition_id_tensor())
            outs = bass2jax._bass_exec_p.bind(
                *operands,
                out_avals=tuple(out_avals),
                in_names=tuple(all_in_names),
                out_names=tuple(out_names),
                lowering_input_output_aliases=(),
                sim_require_finite=True,
                sim_require_nnan=True,
                nc=nc,
            )
            return tuple(outs)

        devices = jax.devices()[:NCORES]
        mesh = Mesh(np.asarray(devices), ("core",))
        self.mesh = mesh
        self.pspec = PartitionSpec("core")
        in_specs = (PartitionSpec("core"),) * (n_params + n_outs)
        out_specs = (PartitionSpec("core"),) * n_outs
        self.donate = tuple(range(n_params, n_params + n_outs))
        self.sharded = jax.jit(
            shard_map(_body, mesh=mesh, in_specs=in_specs, out_specs=out_specs,
                      check_rep=False),
            donate_argnums=self.donate, keep_unused=True)

    def concat_inputs(self, in_maps):
        return [np.concatenate([np.asarray(m[name]) for m in in_maps], axis=0)
                for name in self.in_names]

    def zeros(self):
        return [np.zeros((NCORES * s[0], *s[1:]), d) for (s, d) in self.zero_shapes]

    def run(self, concat_in):
        outs = self.sharded(*concat_in, *self.zeros())
        outs = self.jax.block_until_ready(outs)
        return outs

    def device_inputs(self, concat_in):
        from jax.sharding import NamedSharding
        sh = NamedSharding(self.mesh, self.pspec)
        return [self.jax.device_put(a, sh) for a in concat_in]

    def device_zeros(self):
        import jax.numpy as jnp
        from jax.sharding import NamedSharding
        sh = NamedSharding(self.mesh, self.pspec)
        return [jnp.zeros((NCORES * s[0], *s[1:]), d, device=sh)
                for (s, d) in self.zero_shapes]

    def run_device(self, dev_in):
        outs = self.sharded(*dev_in, *self.device_zeros())
        outs = self.jax.block_until_ready(outs)
        return outs

    def split_out(self, outs):
        res = {}
        for i, name in enumerate(self.out_names):
            res[name] = np.asarray(outs[i]).reshape(NCORES, *self.out_avals[i].shape)
        return res


_CACHE = {}


def _get_runner(repeat=1, loop_n=0, parts=("a", "bc"), pre_parts=()):
    key = ("runner", repeat, loop_n, tuple(parts), tuple(pre_parts))
    if key not in _CACHE:
        nc = build_module(repeat=repeat, loop_n=loop_n, parts=parts, pre_parts=pre_parts)
        _CACHE[key] = _Runner(nc)
    return _CACHE[key]


def kernel(x, W_qkv, b_qkv, W_proj, b_proj):
    runner = _get_runner(repeat=1)
    in_maps = _host_prep(x, W_qkv, b_qkv, W_proj)
    concat_in = runner.concat_inputs(in_maps)
    outs = runner.run(concat_in)
    parts = runner.split_out(outs)["out"]  # [8, 4096, 1024] fp16
    full = parts.astype(np.float32).sum(axis=0)
    full = full + np.asarray(b_proj, np.float32)[None, :]
    return full.reshape(B, T, DM)


# revision 32
# speedup vs baseline: 1.2135x; 1.2135x over previous
"""Causal self-attention (b=2, t=2048, d_model=1024, 16 heads) on 8 trn2 cores.

Sharding: tensor-parallel over heads (2 heads per core). Each core computes
qkv = x @ W_qkv[:, head-slice], attention for its heads, and a partial
out_heads @ W_proj[head-rows, :]. The 8 partial [4096, 1024] fp16 outputs are
summed on the host (the all-reduce after proj), plus b_proj.

Device pipeline (per 512-row chunk, engines kept co-busy):
- Stage A: Q^T/K^T = W.T @ xT on PE (column-major, heads stacked on
  partitions); V computed ROW-major directly (lhsT = xT tile, rhs = W_v
  k-tile) so no PE transposes / DVE copies are needed — V lands in PSUM as
  [rows, dims] and Pool evacuates it into the 130-wide V blocks (ones column
  per head accumulates the softmax denominator during the AV matmul).
- Scores are computed transposed (sT[k, q]); no max-subtraction needed.
- Causality: k-tiles above the diagonal are skipped; diagonal 128-bands get
  -2000 added in PSUM via a small accumulate-matmul (exp then gives 0),
  removing the DVE mask-multiply from the exp->AV critical chain.
- Softmax normalize: one DVE reciprocal of the accumulated denominator row,
  one PE broadcast matmul (ones[1,128] lhsT), two DVE multiplies -> ot.
- Proj per 128-row tile into PSUM; Pool evacuates to fp16; one DMA per
  512-row chunk writes the fp16 partial to HBM (host sums partials in f32).
- PSUM budget: 2 x [128,1024] score/proj slots + 2 x [65,1024] AV-accum
  slots = 8 banks. Normalize+proj of chunk c are emitted interleaved into
  chunk c+1's k-loop so PE/ACT never drain.
"""

import sys

sys.path.insert(0, "/opt/trn_rl_repo")

import numpy as np

import concourse.bass as bass  # noqa: F401
import concourse.tile as tile
from concourse import bacc, mybir

F32 = mybir.dt.float32
F32R = mybir.dt.float32r
F16 = mybir.dt.float16
F8 = mybir.dt.float8e4
BF16 = mybir.dt.bfloat16
DT_AT = BF16   # operand dtype for stage A matmuls and attention (B/C)
EXP = mybir.ActivationFunctionType.Exp
IDENT = mybir.ActivationFunctionType.Identity

B = 2
T = 2048
DM = 1024
NH = 16
HD = 64
ROWS = B * T            # 4096
NCORES = 8
HPC = NH // NCORES      # heads per core = 2
WCOLS = HPC * HD        # 128 qkv columns per core for each of q/k/v
QCH = 512               # query chunk
KTILE = 128             # key tile
NQC = T // QCH          # 4 query chunks per batch
NKT_B = T // KTILE      # 16 key tiles per batch
NRC = ROWS // QCH       # 8 row chunks
NKD = DM // 128         # 8 d_model k-tiles
VW = 2 * (HD + 1)       # 130: V block width (2 heads x (64 dims + ones col))
MASKV = -2000.0         # pre-scale causal mask add; exp(0.125*-2000) == 0


class _Alloc:
    """Tag-based routing to the right tile pool."""
    WORK = {"xt", "ea", "osb", "rc2", "bc", "ea8"}
    WORK_BUFS = {"xt": 3, "ea": 8, "osb": 3, "rc2": 2, "bc": 2, "ea8": 4}

    def __init__(self, pers, work, ps):
        self.pers, self.work, self.ps = pers, work, ps

    def tile(self, shape, dt, tag):
        if tag in ("ps2", "pso"):
            return self.ps.tile(shape, dt, tag=tag, name=tag, bufs=2)
        if tag in self.WORK:
            return self.work.tile(shape, dt, tag=tag, name=tag, bufs=self.WORK_BUFS[tag])
        return self.pers.tile(shape, dt, tag=tag, name=tag)


def _emit_consts(nc, al, aps):
    (xt_d, wq_d, wk_d, wv_d, wp_d, wpb_d, bq_d, bk_d, bvr_d, caus_d, causf_d,
     vones8_d, onesb_d, nl8_d, onesr_d, id_d, vones_d, out_d) = aps
    C = {}
    C["qt"] = al.tile([128, ROWS], DT_AT, tag="qt")
    C["kt"] = al.tile([128, ROWS], DT_AT, tag="kt")
    C["v"] = al.tile([128, (ROWS // 128) * VW], DT_AT, tag="v")
    C["ot"] = al.tile([128, ROWS], F32R, tag="ot")
    C["wq"] = al.tile([128, DM], DT_AT, tag="wq")
    C["wk"] = al.tile([128, DM], DT_AT, tag="wk")
    C["wv"] = al.tile([128, DM], DT_AT, tag="wv")
    C["wp"] = al.tile([128, DM], F32R, tag="wp")
    C["wpb"] = al.tile([128, DM], DT_AT, tag="wpb")
    C["otb"] = al.tile([128, ROWS], DT_AT, tag="otb")
    C["bq"] = al.tile([128, 1], F32, tag="bq")
    C["bk"] = al.tile([128, 1], F32, tag="bk")
    C["bvr"] = al.tile([1, 128], DT_AT, tag="bvr")
    C["caus"] = al.tile([128, 128], DT_AT, tag="caus")
    C["causf"] = al.tile([128, 128], DT_AT, tag="causf")
    C["v8a"] = al.tile([128, (ROWS // 128) * (HD + 1)], F8, tag="v8a")
    C["v8b"] = al.tile([128, (ROWS // 128) * (HD + 1)], F8, tag="v8b")
    C["ones_b"] = al.tile([1, 128], DT_AT, tag="ones_b")
    C["nl8"] = al.tile([128, 1], F32, tag="nl8")
    C["ones_r"] = al.tile([1, 128], F32R, tag="ones_r")
    C["id"] = al.tile([128, 128], DT_AT, tag="id")
    for k in range(NKD):
        nc.sync.dma_start(C["wq"][:, k * 128:(k + 1) * 128], wq_d[k * 128:(k + 1) * 128, :])
        nc.sync.dma_start(C["wk"][:, k * 128:(k + 1) * 128], wk_d[k * 128:(k + 1) * 128, :])
        nc.sync.dma_start(C["wv"][:, k * 128:(k + 1) * 128], wv_d[k * 128:(k + 1) * 128, :])
    nc.sync.dma_start(C["wp"][:], wp_d[:])
    nc.sync.dma_start(C["wpb"][:], wpb_d[:])
    nc.sync.dma_start(C["bq"][:], bq_d[:])
    nc.sync.dma_start(C["bk"][:], bk_d[:])
    nc.sync.dma_start(C["bvr"][:], bvr_d[:])
    nc.sync.dma_start(C["caus"][:], caus_d[:])
    nc.sync.dma_start(C["causf"][:], causf_d[:])
    v8a_blocks = C["v8a"].rearrange("p (i w) -> p i w", w=HD + 1)
    v8b_blocks = C["v8b"].rearrange("p (i w) -> p i w", w=HD + 1)
    nc.sync.dma_start(v8a_blocks[:, :, HD], vones8_d[:])
    nc.sync.dma_start(v8b_blocks[:, :, HD], vones8_d[:])
    nc.sync.dma_start(C["ones_b"][:], onesb_d[:])
    nc.sync.dma_start(C["nl8"][:], nl8_d[:])
    nc.sync.dma_start(C["ones_r"][:], onesr_d[:])
    nc.sync.dma_start(C["id"][:], id_d[:])
    # ones columns of the V blocks (cols 64 and 129 of each 130-block)
    v_blocks = C["v"].rearrange("p (i w) -> p i w", w=VW)
    nc.sync.dma_start(v_blocks[:, :, HD], vones_d[:])
    nc.sync.dma_start(v_blocks[:, :, 2 * HD + 1], vones_d[:])
    return C


def _emit_stage_a(nc, al, aps, C, opts=frozenset()):
    (xt_d, wq_d, wk_d, wv_d, wp_d, wpb_d, bq_d, bk_d, bvr_d, caus_d, causf_d,
     vones8_d, onesb_d, nl8_d, onesr_d, id_d, vones_d, out_d) = aps
    qt_sb, kt_sb, v_sb = C["qt"], C["kt"], C["v"]
    wq_sb, wk_sb, wv_sb = C["wq"], C["wk"], C["wv"]

    for rc in range(NRC):
        cs = rc * QCH
        xt_t = al.tile([128, NKD * QCH], DT_AT, tag="xt")
        xt_v = xt_t.rearrange("p (k q) -> p k q", q=QCH)
        if "noxt" not in opts:
            nc.sync.dma_start(
                xt_v,
                xt_d[:, cs:cs + QCH].rearrange("(k p) q -> p k q", p=128))
        qk = al.tile([128, 2 * QCH], F32, tag="ps2")
        psq = qk[:, 0:QCH]
        psk = qk[:, QCH:2 * QCH]
        psv = al.tile([128, QCH], F32, tag="pso")
        # The 4 V row-blocks are independent sub-bank accumulations; PSUM
        # allows only one start-group per 2KB zero region, so pre-zero the
        # bank and accumulate everything with start=False.
        nc.vector.memzero(psv[:])
        for k in range(NKD):
            st = (k == 0)
            sp = (k == NKD - 1)
            nc.tensor.matmul(psq, wq_sb[:, k * 128:(k + 1) * 128], xt_v[:, k, :], start=st, stop=sp)
            nc.tensor.matmul(psk, wk_sb[:, k * 128:(k + 1) * 128], xt_v[:, k, :], start=st, stop=sp)
            for i in range(QCH // 128):
                nc.tensor.matmul(psv[:, i * 128:(i + 1) * 128],
                                 xt_v[:, k, i * 128:(i + 1) * 128],
                                 wv_sb[:, k * 128:(k + 1) * 128],
                                 start=False, stop=False, skip_group_check=True)
        for i in range(QCH // 128):
            nc.tensor.matmul(psv[:, i * 128:(i + 1) * 128], C["ones_b"][:],
                             C["bvr"][:], start=False, stop=False,
                             skip_group_check=True)
        nc.scalar.activation(qt_sb[:, cs:cs + QCH], psq, IDENT, bias=C["bq"][:])
        nc.scalar.activation(kt_sb[:, cs:cs + QCH], psk, IDENT, bias=C["bk"][:])
        VW8 = HD + 1
        for i in range(QCH // 128):
            blk = rc * (QCH // 128) + i
            if "fp8av" in opts:
                dst_a = C["v8a"][:, blk * VW8:blk * VW8 + HD]
                dst_b = C["v8b"][:, blk * VW8:blk * VW8 + HD]
            else:
                dst_a = v_sb[:, blk * VW:blk * VW + HD]
                dst_b = v_sb[:, blk * VW + HD + 1:blk * VW + 2 * HD + 1]
            if "vact" in opts:
                nc.scalar.activation(dst_a, psv[:, i * 128:i * 128 + HD], IDENT)
                nc.scalar.activation(dst_b, psv[:, i * 128 + HD:(i + 1) * 128], IDENT)
            else:
                nc.vector.tensor_copy(dst_a, psv[:, i * 128:i * 128 + HD])
                nc.vector.tensor_copy(dst_b, psv[:, i * 128 + HD:(i + 1) * 128])


def _emit_norm(nc, al, C, qglob, pso2, opts=frozenset()):
    """Softmax normalize chunk at qglob: ot[:, qglob:+QCH] = pso/denom."""
    ot_sb = C["otb"] if "otbf" in opts else C["ot"]
    rcv = al.tile([1, 2 * QCH], F32R, tag="rc2")
    with nc.allow_low_precision(reason="f32r softmax denom recip"):
        nc.vector.reciprocal(rcv[:], pso2[HD:HD + 1, :])
    bc2 = al.tile([HD, 2 * QCH], F32, tag="bc")
    if "nobc" in opts:
        nc.gpsimd.memset(bc2[:], 1.0)  # timing ablation: skip real broadcast
    elif "dmabc" in opts:
        # broadcast the reciprocal row across 64 partitions via DMA (keeps
        # PE/ACT out of the normalize chain)
        nc.sync.dma_start(bc2[:], rcv.bitcast(F32).partition_broadcast(HD)[:, 0, :])
    else:
        psbc = al.tile([128, 2 * QCH], F32, tag="ps2")
        nc.tensor.matmul(psbc[0:HD, 0:QCH], C["ones_r"][:, 0:HD], rcv[:, 0:QCH])
        nc.tensor.matmul(psbc[0:HD, QCH:2 * QCH], C["ones_r"][:, 0:HD],
                         rcv[:, QCH:2 * QCH])
        nc.scalar.activation(bc2[:], psbc[0:HD, :], IDENT)
    nc.vector.tensor_mul(ot_sb[0:HD, qglob:qglob + QCH], pso2[0:HD, 0:QCH],
                         bc2[:, 0:QCH])
    nc.vector.tensor_mul(ot_sb[HD:128, qglob:qglob + QCH], pso2[0:HD, QCH:2 * QCH],
                         bc2[:, QCH:2 * QCH])


def _emit_proj(nc, al, aps, C, qglob, j, osb, opts=frozenset()):
    """Proj one 128-row q-tile j of the chunk at qglob into osb (fp16)."""
    if "otbf" in opts:
        ot_sb, wp_sb = C["otb"], C["wpb"]
    else:
        ot_sb, wp_sb = C["ot"], C["wp"]
    q0 = qglob + j * 128
    psp = al.tile([128, 2 * QCH], F32, tag="ps2")
    for ct in range(DM // 512):
        nc.tensor.matmul(psp[:, ct * 512:(ct + 1) * 512], ot_sb[:, q0:q0 + 128],
                         wp_sb[:, ct * 512:(ct + 1) * 512])
    if "noosb" in opts:
        return
    if j % 2 == 0:
        nc.scalar.activation(osb[:, j * DM:(j + 1) * DM], psp[:], IDENT)
    else:
        nc.vector.tensor_copy(osb[:, j * DM:(j + 1) * DM], psp[:])


def _emit_outdma(nc, aps, qglob, osb):
    out_d = aps[-1]
    nc.gpsimd.dma_start(
        out_d[qglob:qglob + QCH, :].rearrange("(j p) d -> p j d", p=128),
        osb.rearrange("p (j d) -> p j d", d=DM))


def _emit_attn(nc, al, aps, C, opts=frozenset()):
    (xt_d, wq_d, wk_d, wv_d, wp_d, wpb_d, bq_d, bk_d, bvr_d, caus_d, causf_d,
     vones8_d, onesb_d, nl8_d, onesr_d, id_d, vones_d, out_d) = aps
    qt_sb, kt_sb, v_sb = C["qt"], C["kt"], C["v"]
    caus_sb, id_sb = C["caus"], C["id"]

    NJ = QCH // 128  # 4 proj tiles per chunk

    # Two-chunk software pipeline: normalize of chunk c-1 and projection of
    # chunk c-2 are interleaved into chunk c's k-loop. By the time proj(c-2)
    # matmuls reach PE, ot(c-2) has been ready for a whole chunk, so the
    # recip->bcast->mul chain latency never gates the PE queue.
    def make_steps(h1, h2):
        steps = []
        if h1 is not None and "nonorm" not in opts and "noproj" not in opts:
            steps.append(("norm", h1))
        if h2 is not None and "noproj" not in opts:
            steps += [("proj", h2, j) for j in range(NJ)]
        return steps

    def emit_step(st):
        if st[0] == "norm":
            qg, pso2, osb = st[1]
            _emit_norm(nc, al, C, qg, pso2, opts)
        else:
            qg, pso2, osb = st[1]
            j = st[2]
            _emit_proj(nc, al, aps, C, qg, j, osb, opts)
            if j == NJ - 1 and "noout" not in opts and "noosb" not in opts:
                _emit_outdma(nc, aps, qg, osb)

    hist = [None, None]
    for b in range(B):
        # Descending qc: every chunk's deferred norm/proj lands in a
        # successor with a long k-loop, and the end-of-iteration drain tail
        # is the shortest (4-tile) chunk.
        for qc in reversed(range(NQC)):
            qglob = b * T + qc * QCH
            nkt = (qc + 1) * (QCH // KTILE)
            pso2 = al.tile([HD + 1, 2 * QCH], F32, tag="pso")
            osb = al.tile([128, NJ * DM], F16, tag="osb")
            steps = make_steps(hist[0], hist[1])
            step = 0
            if "fp8av" in opts:
                # Pair-based k-loop: exp scores for two adjacent k-tiles into
                # one fp8 tile, then one DoubleRow AV matmul per head
                # contracts both tiles at once (0.5 cycles/row).
                ndg = QCH // KTILE
                pairs = [(qc * ndg + 2 * j, True) for j in range(ndg // 2)] + \
                        [(2 * j, False) for j in range(qc * ndg // 2)]
                idx = 0
                for pi, (kte, isdg) in enumerate(pairs):
                    sp_ = max(0, kte * KTILE - qc * QCH)  # pair column start
                    ea8 = al.tile([128, 4 * QCH], F8, tag="ea8")
                    ea8v = ea8.rearrange("p (i h q) -> p i h q", i=2, h=2)
                    for par in range(2):
                        kt = kte + par
                        r = kt * KTILE - qc * QCH
                        i = b * NKT_B + kt
                        kcol = b * T + kt * KTILE
                        ps2 = al.tile([128, 2 * QCH], F32, tag="ps2")
                        nc.tensor.matmul(ps2[:, sp_:QCH],
                                         kt_sb[0:HD, kcol:kcol + KTILE],
                                         qt_sb[0:HD, qglob + sp_:qglob + QCH],
                                         start=True, stop=not isdg)
                        nc.tensor.matmul(ps2[:, QCH + sp_:],
                                         kt_sb[HD:128, kcol:kcol + KTILE],
                                         qt_sb[HD:128, qglob + sp_:qglob + QCH],
                                         start=True, stop=not isdg)
                        if isdg:
                            # triangular band at this tile's own diagonal
                            for off in (0, QCH):
                                if par == 1:  # fully-masked 128 cols below
                                    nc.tensor.matmul(
                                        ps2[:, off + sp_:off + sp_ + KTILE],
                                        C["causf"][:], id_sb[:],
                                        start=False, stop=False)
                                nc.tensor.matmul(
                                    ps2[:, off + r:off + r + KTILE],
                                    caus_sb[:], id_sb[:],
                                    start=False, stop=True)
                        src_v = ps2.rearrange("p (h q) -> p h q", h=2)[:, :, sp_:]
                        dst_v = ea8v[:, par, :, sp_:]
                        # -ln4 bias keeps exp outputs inside fp8-e4m3 range
                        # (max logit ~7 -> e^7/4 = 274 < 448) while limiting
                        # denormal quantization of small weights; the softmax
                        # denominator scales identically so it cancels.
                        nc.scalar.activation(dst_v, src_v, EXP, scale=0.125,
                                             bias=C["nl8"][:])
                    st = (pi == 0)
                    sp2 = (pi == len(pairs) - 1)
                    blk = b * NKT_B + kte
                    VW8 = HD + 1
                    for h, v8 in ((0, C["v8a"]), (1, C["v8b"])):
                        nc.tensor.matmul(
                            pso2[:, h * QCH + sp_:(h + 1) * QCH],
                            v8[:, blk * VW8:(blk + 2) * VW8].rearrange(
                                "p (i m) -> p i m", i=2),
                            ea8v[:, :, h, sp_:],
                            start=st, stop=sp2,
                            perf_mode=mybir.MatmulPerfMode.DoubleRow)
                    for _ in range(2):
                        if idx >= 1 and step < len(steps):
                            emit_step(steps[step])
                            step += 1
                        idx += 1
                while step < len(steps):
                    emit_step(steps[step])
                    step += 1
                hist = [(qglob, pso2, osb), hist[0]]
                continue
            # Diagonal (small, latency-bound) k-tiles first: their
            # sc->mask->exp->AV chains overlap the deferred norm/proj of the
            # previous chunk; the big full tiles then stream PE-dense.
            kts = list(range(qc * (QCH // KTILE), nkt)) + list(range(qc * (QCH // KTILE)))
            for idx, kt in enumerate(kts):
                r = kt * KTILE - qc * QCH
                s = max(0, r)          # valid column suffix start
                i = b * NKT_B + kt     # global 128-row tile index for K/V
                kcol = b * T + kt * KTILE
                ps2 = al.tile([128, 2 * QCH], F32, tag="ps2")
                diag = r >= 0 and "nomask" not in opts
                nc.tensor.matmul(ps2[:, s:QCH], kt_sb[0:HD, kcol:kcol + KTILE],
                                 qt_sb[0:HD, qglob + s:qglob + QCH],
                                 start=True, stop=not diag)
                nc.tensor.matmul(ps2[:, QCH + s:], kt_sb[HD:128, kcol:kcol + KTILE],
                                 qt_sb[HD:128, qglob + s:qglob + QCH],
                                 start=True, stop=not diag)
                if diag and "nomask" not in opts:
                    # add -2000 to the upper-triangular 128-band
                    nc.tensor.matmul(ps2[:, s:s + KTILE], caus_sb[:], id_sb[:],
                                     start=False, stop=True)
                    nc.tensor.matmul(ps2[:, QCH + s:QCH + s + KTILE], caus_sb[:],
                                     id_sb[:], start=False, stop=True)
                ea2 = al.tile([128, 2 * QCH], DT_AT, tag="ea")
                src_v = ps2.rearrange("p (h q) -> p h q", h=2)[:, :, s:]
                dst_v = ea2.rearrange("p (h q) -> p h q", h=2)[:, :, s:]
                if "noexp" in opts:
                    nc.scalar.activation(dst_v, src_v, IDENT, scale=1e-9)
                else:
                    nc.scalar.activation(dst_v, src_v, EXP, scale=0.125)
                st = (idx == 0)
                sp = (idx == nkt - 1)
                nc.tensor.matmul(pso2[:, s:QCH], v_sb[:, i * VW:i * VW + HD + 1],
                                 ea2[:, s:QCH], start=st, stop=sp)
                nc.tensor.matmul(pso2[:, QCH + s:], v_sb[:, i * VW + HD + 1:i * VW + VW],
                                 ea2[:, QCH + s:], start=st, stop=sp)
                if idx >= 1 and step < len(steps):
                    emit_step(steps[step])
                    step += 1
            while step < len(steps):
                emit_step(steps[step])
                step += 1
            hist = [(qglob, pso2, osb), hist[0]]
    # flush: norm of the last chunk, then proj of the last two chunks
    for st in make_steps(hist[0], hist[1]):
        emit_step(st)
    for st in make_steps(None, hist[0]):
        emit_step(st)


def _emit_body(nc, al, aps, C, parts=("a", "bc")):
    opts = set(parts)
    if "a" in parts:
        _emit_stage_a(nc, al, aps, C, opts)
    if "bc" in parts:
        _emit_attn(nc, al, aps, C, opts)


def build_module(repeat=1, loop_n=0, parts=("a", "bc"), pre_parts=()):
    nc = bacc.Bacc("TRN2", target_bir_lowering=False, debug=False,
                   enable_asserts=True, num_devices=NCORES)

    def din(name, shape, dt):
        return nc.dram_tensor(name, shape, dt, kind="ExternalInput").ap()

    aps = (
        din("xt", [DM, ROWS], DT_AT),
        din("wq", [DM, WCOLS], DT_AT),
        din("wk", [DM, WCOLS], DT_AT),
        din("wv", [DM, WCOLS], DT_AT),
        din("wp", [WCOLS, DM], F32R),
        din("wpb", [WCOLS, DM], DT_AT),
        din("bq", [WCOLS, 1], F32),
        din("bk", [WCOLS, 1], F32),
        din("bvr", [1, WCOLS], DT_AT),
        din("caus", [128, 128], DT_AT),
        din("causf", [128, 128], DT_AT),
        din("vones8", [128, ROWS // 128], F8),
        din("ones_b", [1, 128], DT_AT),
        din("nl8", [128, 1], F32),
        din("ones_r", [1, 128], F32R),
        din("ident", [128, 128], DT_AT),
        din("vones", [128, ROWS // 128], DT_AT),
        nc.dram_tensor("out", [ROWS, DM], F16, kind="ExternalOutput").ap(),
    )
    with tile.TileContext(nc) as tc:
        with tc.tile_pool(name="pers", bufs=1) as pers, \
             tc.tile_pool(name="work", bufs=2) as work, \
             tc.tile_pool(name="ps", bufs=2, space="PSUM") as psp:
            al = _Alloc(pers, work, psp)
            al.tc = tc
            consts = _emit_consts(nc, al, aps)
            if pre_parts:
                _emit_body(nc, al, aps, consts, parts=pre_parts)
            if loop_n:
                with tc.For_i(0, loop_n, 1):
                    for r in range(repeat):
                        _emit_body(nc, al, aps, consts, parts=parts)
            else:
                for r in range(repeat):
                    _emit_body(nc, al, aps, consts, parts=parts)
    nc.compile()
    return nc


def _host_prep(x, W_qkv, b_qkv, W_proj):
    import ml_dtypes
    bf16 = ml_dtypes.bfloat16
    x = np.asarray(x, np.float32)
    W_qkv = np.asarray(W_qkv, np.float32)
    b_qkv = np.asarray(b_qkv, np.float32)
    W_proj = np.asarray(W_proj, np.float32)
    xt = np.ascontiguousarray(x.reshape(ROWS, DM).T.astype(bf16))
    caus = np.triu(np.full((128, 128), MASKV, np.float32), 1).astype(bf16)
    f8 = mybir.dt.np(mybir.dt.float8e4)
    causf = np.full((128, 128), MASKV, np.float32).astype(bf16)
    vones8 = np.ones((128, ROWS // 128), f8)
    ident = np.eye(128, dtype=bf16)
    in_maps = []
    for c in range(NCORES):
        h0 = c * WCOLS  # first qkv column of this core's 2 heads
        in_maps.append({
            "xt": xt,
            "wq": np.ascontiguousarray(W_qkv[:, h0:h0 + WCOLS].astype(bf16)),
            "wk": np.ascontiguousarray(W_qkv[:, DM + h0:DM + h0 + WCOLS].astype(bf16)),
            "wv": np.ascontiguousarray(W_qkv[:, 2 * DM + h0:2 * DM + h0 + WCOLS].astype(bf16)),
            "wp": np.ascontiguousarray(W_proj[h0:h0 + WCOLS, :]),
            "wpb": np.ascontiguousarray(W_proj[h0:h0 + WCOLS, :].astype(bf16)),
            "bq": np.ascontiguousarray(b_qkv[h0:h0 + WCOLS, None]),
            "bk": np.ascontiguousarray(b_qkv[DM + h0:DM + h0 + WCOLS, None]),
            "bvr": np.ascontiguousarray(
                b_qkv[2 * DM + h0:2 * DM + h0 + WCOLS][None, :].astype(bf16)),
            "caus": caus,
            "causf": causf,
            "vones8": vones8,
            "ones_b": np.ones((1, 128), bf16),
            "nl8": np.full((128, 1), -np.log(4.0), np.float32),
            "ones_r": np.ones((1, 128), np.float32),
            "ident": ident,
            "vones": np.ones((128, ROWS // 128), bf16),
        })
    return in_maps


class _Runner:
    """Compile once, execute many times (mirrors bass2jax.run_bass_via_pjrt)."""

    def __init__(self, nc):
        import jax
        from jax.sharding import Mesh, PartitionSpec
        from jax.experimental.shard_map import shard_map
        from concourse import bass2jax
        from concourse import mybir as _mybir

        bass2jax.install_neuronx_cc_hook()
        self.jax = jax
        in_names, out_names, out_avals, zero_shapes = [], [], [], []
        partition_name = nc.partition_id_tensor.name if nc.partition_id_tensor else None
        for alloc in nc.m.functions[0].allocations:
            if not isinstance(alloc, _mybir.MemoryLocationSet):
                continue
            name = alloc.memorylocations[0].name
            if alloc.kind == "ExternalInput":
                if name != partition_name:
                    in_names.append(name)
            elif alloc.kind == "ExternalOutput":
                shape = tuple(alloc.tensor_shape)
                dtype = _mybir.dt.np(alloc.dtype)
                out_names.append(name)
                out_avals.append(jax.core.ShapedArray(shape, dtype))
                zero_shapes.append((shape, dtype))
        self.in_names = in_names
        self.out_names = out_names
        self.out_avals = out_avals
        self.zero_shapes = zero_shapes
        n_params = len(in_names)
        n_outs = len(out_avals)
        all_in_names = in_names + out_names + ([partition_name] if partition_name else [])

        def _body(*args):
            operands = list(args)
            if partition_name is not None:
                operands.append(bass2jax.partition_id_tensor())
            outs = bass2jax._bass_exec_p.bind(
                *operands,
                out_avals=tuple(out_avals),
                in_names=tuple(all_in_names),
                out_names=tuple(out_names),
                lowering_input_output_aliases=(),
                sim_require_finite=True,
                sim_require_nnan=True,
                nc=nc,
            )
            return tuple(outs)

        devices = jax.devices()[:NCORES]
        mesh = Mesh(np.asarray(devices), ("core",))
        self.mesh = mesh
        self.pspec = PartitionSpec("core")
        in_specs = (PartitionSpec("core"),) * (n_params + n_outs)
        out_specs = (PartitionSpec("core"),) * n_outs
        self.donate = tuple(range(n_params, n_params + n_outs))
        self.sharded = jax.jit(
            shard_map(_body, mesh=mesh, in_specs=in_specs, out_specs=out_specs,
                      check_rep=False),
            donate_argnums=self.donate, keep_unused=True)

    def concat_inputs(self, in_maps):
        return [np.concatenate([np.asarray(m[name]) for m in in_maps], axis=0)
                for name in self.in_names]

    def zeros(self):
        return [np.zeros((NCORES * s[0], *s[1:]), d) for (s, d) in self.zero_shapes]

    def run(self, concat_in):
        outs = self.sharded(*concat_in, *self.zeros())
        outs = self.jax.block_until_ready(outs)
        return outs

    def device_inputs(self, concat_in):
        from jax.sharding import NamedSharding
        sh = NamedSharding(self.mesh, self.pspec)
        return [self.jax.device_put(a, sh) for a in concat_in]

    def device_zeros(self):
        import jax.numpy as jnp
        from jax.sharding import NamedSharding
        sh = NamedSharding(self.mesh, self.pspec)
        return [jnp.zeros((NCORES * s[0], *s[1:]), d, device=sh)
                for (s, d) in self.zero_shapes]

    def run_device(self, dev_in):
        outs = self.sharded(*dev_in, *self.device_zeros())
        outs = self.jax.block_until_ready(outs)
        return outs

    def split_out(self, outs):
        res = {}
        for i, name in enumerate(self.out_names):
            res[name] = np.asarray(outs[i]).reshape(NCORES, *self.out_avals[i].shape)
        return res


_CACHE = {}


def _get_runner(repeat=1, loop_n=0, parts=("a", "bc"), pre_parts=()):
    key = ("runner", repeat, loop_n, tuple(parts), tuple(pre_parts))
    if key not in _CACHE:
        nc = build_module(repeat=repeat, loop_n=loop_n, parts=parts, pre_parts=pre_parts)
        _CACHE[key] = _Runner(nc)
    return _CACHE[key]


def kernel(x, W_qkv, b_qkv, W_proj, b_proj):
    runner = _get_runner(repeat=1)
    in_maps = _host_prep(x, W_qkv, b_qkv, W_proj)
    concat_in = runner.concat_inputs(in_maps)
    outs = runner.run(concat_in)
    parts = runner.split_out(outs)["out"]  # [8, 4096, 1024] fp16
    full = parts.astype(np.float32).sum(axis=0)
    full = full + np.asarray(b_proj, np.float32)[None, :]
    return full.reshape(B, T, DM)
